# revision 35
# baseline (speedup 1.0000x reference)
"""Trainium2 Bass kernel for DigitsCapsule dynamic routing.

Strategy (8 NeuronCores, data-parallel over batch B=512 -> 64 per core):
  u_hat = einsum('BIk,IklO->BIlO', x, w) is NEVER materialized (264 MB).
  Instead, per routing iteration:
    s    = x @ (e ⊙ w)              (PE matmul over r=(k,I)=9216, e = unnorm. softmax weights)
    v    = squash(s / S[l])          (softmax normalizer folded into squash)
    T2   = xᵀ @ v                    (PE outer-product accumulation)
    u_vj = Σ_{k,O} w ⊙ T2            (DVE mult + grouped reduce)  -> [I,l] agreement
    b   += AllReduce(u_vj)           (mean over full B via 8-core collective)
  Last iteration skips the u_vj/collective (dead in the reference).

Row space r = k*1152 + I (k-major); free space f = l*7 + O (w's natural order).
All layout permutes are done host-side in numpy; the device program has zero
transposes.
"""

import numpy as np

B, I, K, L, O = 512, 1152, 8, 16, 7
NC = 8
BL = B // NC          # 64 batch rows per core
R = K * I             # 9216
F = L * O             # 112
NI = I // 128         # 9 partition chunks of I
ITERS = 3

_CACHE = {}


def _build(dt_key, repeat=1, abl=()):
    """abl: ablation flags for benchmarking — subsets of
    {"no_ar", "no_u", "no_wc", "no_smm"}."""
    import concourse.bacc as bacc
    import concourse.mybir as mybir
    import concourse.tile as tile

    DT = {"f32": mybir.dt.float32, "f16": mybir.dt.float16}[dt_key]
    F32 = mybir.dt.float32
    AF = mybir.ActivationFunctionType
    ALU = mybir.AluOpType
    AX = mybir.AxisListType

    nc = bacc.Bacc("TRN2", target_bir_lowering=False, debug=False, num_devices=NC)

    x_nat_d = nc.dram_tensor("x_nat", [BL, R], DT, kind="ExternalInput")
    x_T_d = nc.dram_tensor("x_T", [R, BL], DT, kind="ExternalInput")
    w2_d = nc.dram_tensor("w2", [I, K * F], DT, kind="ExternalInput")
    y_d = nc.dram_tensor("y", [BL, O, L], F32, kind="ExternalOutput")

    with tile.TileContext(nc) as tc:
        with (
            tc.tile_pool(name="const", bufs=1) as cpool,
            tc.tile_pool(name="work", bufs=2) as wpool,
            tc.tile_pool(name="wc", bufs=10) as wcfull,
            tc.tile_pool(name="step6", bufs=4) as wcpool,
            tc.tile_pool(name="small", bufs=2) as spool,
            tc.tile_pool(name="ps_s", bufs=1, space="PSUM") as ps_s,
            tc.tile_pool(name="ps_t2", bufs=6, space="PSUM") as ps_t2,
            tc.tile_pool(name="ps_sm", bufs=1, space="PSUM") as ps_sm,
            tc.tile_pool(name="dram", bufs=2, space="DRAM") as dpool,
        ):
            # ---- load inputs (issue spread across engine DGE queues) ----
            x_nat = cpool.tile([BL, R], DT, tag="x_nat")
            for h in range(2):
                nc.gpsimd.dma_start(x_nat[:, h * R // 2:(h + 1) * R // 2],
                                    x_nat_d[:, h * R // 2:(h + 1) * R // 2])

            # xT tiles: slot t=(k*9+i) holds rows k*1152+i*128 .. +128
            NT = K * NI
            xT = cpool.tile([128, NT * BL], DT, tag="xT")
            xt_src = x_T_d[:].rearrange("(t p) b -> p t b", p=128)
            xt_dst = xT[:].rearrange("p (t b) -> p t b", t=NT)
            for h in range(4):
                lo, hi = h * NT // 4, (h + 1) * NT // 4
                nc.sync.dma_start(xt_dst[:, lo:hi], xt_src[:, lo:hi])

            w2 = cpool.tile([128, NI * K * F], DT, tag="w2")
            w2_src = w2_d[:].rearrange("(i p) f -> p i f", p=128)
            w2_dst = w2[:].rearrange("p (i f) -> p i f", i=NI)
            for h in range(3):
                lo, hi = h * 3, (h + 1) * 3
                nc.scalar.dma_start(w2_dst[:, lo:hi], w2_src[:, lo:hi])

            ones = cpool.tile([128, 1], DT, tag="ones")
            nc.vector.memset(ones[:], 1.0)
            ones64 = cpool.tile([1, BL], F32, tag="ones64")
            nc.vector.memset(ones64[:], 1.0)

            b_acc = cpool.tile([128, NI * L], F32, tag="b_acc")

            def w2_i(i):
                return w2[:, i * K * F:(i + 1) * K * F]

            for rep in range(repeat):
             for t in range(ITERS):
                # ---- coupling coefficients (unnormalized) ----
                if t == 0:
                    e9 = None         # e == 1 -> Wc == w2, invS == 1/1152
                    invS_b = None
                else:
                    # e replicated over O (contiguous innermost for Wc mult)
                    e9 = wpool.tile([128, NI * F], DT, tag="e9")
                    nc.scalar.activation(
                        e9[:].rearrange("p (i l o) -> p i l o", i=NI, l=L),
                        b_acc[:].rearrange("p (i l) -> p i l", i=NI)
                        .unsqueeze(3).to_broadcast((128, NI, L, O)),
                        AF.Exp, scale=1.0 / B)
                    # compact e for the column-sum S (N<=512 matmul limit)
                    e_nat = wpool.tile([128, NI * L], DT, tag="e_nat")
                    nc.scalar.activation(e_nat[:], b_acc[:], AF.Exp, scale=1.0 / B)
                    sm_ps = ps_sm.tile([BL, NI * L + L], F32, tag="sm")
                    ssum = sm_ps[0:1, 0:NI * L]
                    nc.tensor.matmul(ssum, ones[:], e_nat[:], start=True, stop=True)
                    S16 = spool.tile([1, L], F32, tag="S16")
                    nc.vector.tensor_reduce(
                        S16[:], ssum.rearrange("p (i l) -> p l i", i=NI),
                        axis=AX.X, op=ALU.add)
                    invS16 = spool.tile([1, L], F32, tag="invS16")
                    nc.vector.reciprocal(invS16[:], S16[:])
                    bc_ps = sm_ps[0:BL, NI * L:NI * L + L]
                    nc.tensor.matmul(bc_ps, ones64[:], invS16[:],
                                     start=True, stop=True)
                    invS_b = spool.tile([BL, L], F32, tag="invS_b")
                    nc.vector.tensor_copy(invS_b[:], bc_ps)

                # ---- s = x @ (e*w) ----
                s_ps = ps_s.tile([BL, F], F32, tag="s_ps")
                if "no_smm" in abl:
                    zz = spool.tile([BL, F], F32, tag="zz")
                    nc.vector.memset(zz[:], 0.0)
                    nc.scalar.activation(s_ps[:], zz[:], AF.Copy)
                wcs = []
                for i in range(NI):
                    if "no_smm" in abl:
                        break
                    if t == 0 or "no_wc" in abl:
                        wcs.append(w2_i(i))
                    else:
                        wc = wcfull.tile([128, K * F], DT, tag="wc")
                        e_b = (e9[:, i * F:(i + 1) * F]
                               .unsqueeze(1).to_broadcast((128, K, F)))
                        nc.vector.tensor_tensor(
                            wc[:].rearrange("p (k f) -> p k f", k=K),
                            w2_i(i).rearrange("p (k f) -> p k f", k=K),
                            e_b, op=ALU.mult)
                        wcs.append(wc[:])
                # k-outer matches the xT DMA arrival order (t = k*9+i)
                if "no_smm" not in abl:
                    for k in range(K):
                        for i in range(NI):
                            tslot = k * NI + i
                            nc.tensor.matmul(
                                s_ps[:],
                                xT[:, tslot * BL:(tslot + 1) * BL],
                                wcs[i][:, k * F:(k + 1) * F],
                                start=(i == 0 and k == 0),
                                stop=(i == NI - 1 and k == K - 1))

                # ---- squash (with 1/S[l] folded in) ----
                s_n = wpool.tile([BL, F], F32, tag="s_n")
                if t == 0:
                    nc.vector.tensor_scalar_mul(s_n[:], s_ps[:], 1.0 / I)
                else:
                    nc.vector.tensor_tensor(
                        s_n[:].rearrange("p (l o) -> p l o", o=O),
                        s_ps[:].rearrange("p (l o) -> p l o", o=O),
                        invS_b[:].unsqueeze(2).to_broadcast((BL, L, O)),
                        op=ALU.mult)
                # squash factor: sq/((1+sq)*sqrt(sq)) == sqrt(sq)/(1+sq)
                sq2 = wpool.tile([BL, F], F32, tag="sq2")
                nc.vector.tensor_tensor(sq2[:], s_n[:], s_n[:], op=ALU.mult)
                sq = spool.tile([BL, L], F32, tag="sq")
                nc.vector.tensor_reduce(
                    sq[:], sq2[:].rearrange("p (l o) -> p l o", o=O),
                    axis=AX.X, op=ALU.add)
                nrm = spool.tile([BL, L], F32, tag="nrm")
                nc.scalar.activation(nrm[:], sq[:], AF.Sqrt)
                d1 = spool.tile([BL, L], F32, tag="d1")
                nc.vector.tensor_scalar_add(d1[:], sq[:], 1.0)
                rin = spool.tile([BL, L], F32, tag="rin")
                nc.vector.reciprocal(rin[:], d1[:])
                fm = spool.tile([BL, L], F32, tag="fm")
                nc.vector.tensor_tensor(fm[:], nrm[:], rin[:], op=ALU.mult)
                # v in fp16 feeds the T2 matmuls directly; final iter in fp32
                vdt = F32 if t == ITERS - 1 else DT
                v_sb = wpool.tile([BL, F], vdt, tag="v_sb")
                nc.vector.tensor_tensor(
                    v_sb[:].rearrange("p (l o) -> p l o", o=O),
                    s_n[:].rearrange("p (l o) -> p l o", o=O),
                    fm[:].unsqueeze(2).to_broadcast((BL, L, O)),
                    op=ALU.mult)

                if t == ITERS - 1:
                    v_out = wpool.tile([BL, F], F32, tag="v_out")
                    nc.vector.tensor_copy(
                        v_out[:].rearrange("p (o l) -> p o l", l=L),
                        v_sb[:].rearrange("p (l o) -> p o l", o=O))
                    nc.sync.dma_start(y_d[:], v_out[:])
                    continue

                # ---- agreement: u = sum_{k,O} w * (x^T v) ----
                if "no_u" in abl:
                    if t == 0:
                        nc.vector.memset(b_acc[:], 0.0)
                    continue
                v16 = v_sb
                u_nat = wpool.tile([128, NI * L], F32, tag="u_nat")
                for i in range(NI):
                    t2s = wcpool.tile([128, K * F], DT, tag="t2s")
                    for h in range(2):
                        t2h = ps_t2.tile([128, 512], F32, tag="t2")
                        for kk in range(4):
                            k = h * 4 + kk
                            nc.tensor.matmul(
                                t2h[:, kk * 128:kk * 128 + F],
                                x_nat[:, k * I + i * 128:k * I + (i + 1) * 128],
                                v16[:], start=True, stop=True)
                        nc.scalar.activation(
                            t2s[:, h * 4 * F:(h + 1) * 4 * F]
                            .rearrange("p (k f) -> p k f", k=4),
                            t2h[:].rearrange("p (k x) -> p k x", k=4)[:, :, 0:F],
                            AF.Copy)
                    prod = wcpool.tile([128, K * F], DT, tag="prod")
                    npool = 3
                    for a in abl:
                        if a.startswith("pool"):
                            npool = int(a[4:])
                    eng = nc.gpsimd if i < npool else nc.vector
                    eng.tensor_tensor(prod[:], t2s[:], w2_i(i), op=ALU.mult)
                    nc.vector.tensor_reduce(
                        u_nat[:, i * L:(i + 1) * L],
                        prod[:].rearrange("p (k l o) -> p l k o", k=K, l=L),
                        axis=AX.XY, op=ALU.add)

                ar_in = dpool.tile([128, NI * L], F32, tag="ar_in")
                ar_out = dpool.tile([128, NI * L], F32, tag="ar_out")
                nc.sync.dma_start(ar_in[:], u_nat[:])
                if "no_ar" in abl:
                    nc.sync.dma_start(ar_out[:], ar_in[:])
                else:
                    nc.gpsimd.collective_compute(
                        "AllReduce", ALU.add,
                        replica_groups=[list(range(NC))],
                        ins=[ar_in.opt()], outs=[ar_out.opt()])
                u_allr = wpool.tile([128, NI * L], F32, tag="u_allr")
                nc.sync.dma_start(u_allr[:], ar_out[:])
                if t == 0:
                    nc.vector.tensor_copy(b_acc[:], u_allr[:])
                else:
                    nc.vector.tensor_add(b_acc[:], b_acc[:], u_allr[:])

    nc.compile()
    return nc


def _get_nc(dt_key, repeat=1, abl=()):
    key = (dt_key, repeat, tuple(sorted(abl)))
    if key not in _CACHE:
        _CACHE[key] = _build(dt_key, repeat, abl)
    return _CACHE[key]


def kernel(x, w, _dt="f16", _trace=False):
    x = np.asarray(x, dtype=np.float32)
    w = np.asarray(w, dtype=np.float32)
    np_dt = {"f32": np.float32, "f16": np.float16}[_dt]

    nc = _get_nc(_dt)

    w2 = np.ascontiguousarray(w.reshape(I, K * F).astype(np_dt))
    in_maps = []
    for c in range(NC):
        xv = x[c * BL:(c + 1) * BL]
        x_nat = np.ascontiguousarray(
            xv.transpose(0, 2, 1).astype(np_dt)).reshape(BL, R)
        x_T = np.ascontiguousarray(
            xv.transpose(2, 1, 0).astype(np_dt)).reshape(R, BL)
        in_maps.append({"x_nat": x_nat, "x_T": x_T, "w2": w2})

    from concourse.bass_utils import run_bass_kernel_spmd
    res = run_bass_kernel_spmd(
        nc, in_maps, core_ids=list(range(NC)), trace=_trace)
    kernel.last_result = res
    out = np.concatenate([res.results[c]["y"] for c in range(NC)], axis=0)
    return out.astype(np.float32)


kernel.last_result = None


# revision 38
# speedup vs baseline: 1.2724x; 1.2724x over previous
"""Trainium2 Bass kernel for DigitsCapsule dynamic routing.

Strategy (8 NeuronCores, data-parallel over batch B=512 -> 64 per core):
  u_hat = einsum('BIk,IklO->BIlO', x, w) is NEVER materialized (264 MB).
  Instead, per routing iteration:
    s    = x @ (e ⊙ w)              (PE matmul over r=(k,I)=9216, e = unnorm. softmax weights)
    v    = squash(s / S[l])          (softmax normalizer folded into squash)
    T2   = xᵀ @ v                    (PE outer-product accumulation)
    u_vj = Σ_{k,O} w ⊙ T2            (DVE mult + grouped reduce)  -> [I,l] agreement
    b   += AllReduce(u_vj)           (mean over full B via 8-core collective)
  Last iteration skips the u_vj/collective (dead in the reference).

Row space r = k*1152 + I (k-major); free space f = l*7 + O (w's natural order).
All layout permutes are done host-side in numpy; the device program has zero
transposes.
"""

import numpy as np

B, I, K, L, O = 512, 1152, 8, 16, 7
NC = 8
BL = B // NC          # 64 batch rows per core
R = K * I             # 9216
F = L * O             # 112
NI = I // 128         # 9 partition chunks of I
ITERS = 3

_CACHE = {}


def _build(dt_key, repeat=1, abl=()):
    """abl: ablation flags for benchmarking — subsets of
    {"no_ar", "no_u", "no_wc", "no_smm"}."""
    import concourse.bacc as bacc
    import concourse.mybir as mybir
    import concourse.tile as tile

    DT = {"f32": mybir.dt.float32, "f16": mybir.dt.float16}[dt_key]
    F32 = mybir.dt.float32
    AF = mybir.ActivationFunctionType
    ALU = mybir.AluOpType
    AX = mybir.AxisListType

    nc = bacc.Bacc("TRN2", target_bir_lowering=False, debug=False, num_devices=NC)

    x_nat_d = nc.dram_tensor("x_nat", [BL, R], DT, kind="ExternalInput")
    x_T_d = nc.dram_tensor("x_T", [R, BL], DT, kind="ExternalInput")
    w2_d = nc.dram_tensor("w2", [I, K * F], DT, kind="ExternalInput")
    y_d = nc.dram_tensor("y", [BL, O, L], F32, kind="ExternalOutput")

    with tile.TileContext(nc) as tc:
        with (
            tc.tile_pool(name="const", bufs=1) as cpool,
            tc.tile_pool(name="work", bufs=2) as wpool,
            tc.tile_pool(name="wc", bufs=10) as wcfull,
            tc.tile_pool(name="step6", bufs=4) as wcpool,
            tc.tile_pool(name="small", bufs=2) as spool,
            tc.tile_pool(name="ps_s", bufs=1, space="PSUM") as ps_s,
            tc.tile_pool(name="ps_t2", bufs=6, space="PSUM") as ps_t2,
            tc.tile_pool(name="ps_sm", bufs=1, space="PSUM") as ps_sm,
            tc.tile_pool(name="dram", bufs=2, space="DRAM") as dpool,
        ):
            # ---- load inputs (issue spread across engine DGE queues) ----
            x_nat = cpool.tile([BL, R], DT, tag="x_nat")
            for h in range(2):
                nc.gpsimd.dma_start(x_nat[:, h * R // 2:(h + 1) * R // 2],
                                    x_nat_d[:, h * R // 2:(h + 1) * R // 2])

            # xT tiles: slot t=(k*9+i) holds rows k*1152+i*128 .. +128
            NT = K * NI
            xT = cpool.tile([128, NT * BL], DT, tag="xT")
            xt_src = x_T_d[:].rearrange("(t p) b -> p t b", p=128)
            xt_dst = xT[:].rearrange("p (t b) -> p t b", t=NT)
            for h in range(4):
                lo, hi = h * NT // 4, (h + 1) * NT // 4
                nc.sync.dma_start(xt_dst[:, lo:hi], xt_src[:, lo:hi])

            w2 = cpool.tile([128, NI * K * F], DT, tag="w2")
            w2_src = w2_d[:].rearrange("(i p) f -> p i f", p=128)
            w2_dst = w2[:].rearrange("p (i f) -> p i f", i=NI)
            for h in range(3):
                lo, hi = h * 3, (h + 1) * 3
                nc.scalar.dma_start(w2_dst[:, lo:hi], w2_src[:, lo:hi])

            ones = cpool.tile([128, 1], DT, tag="ones")
            nc.vector.memset(ones[:], 1.0)
            ones64 = cpool.tile([1, BL], F32, tag="ones64")
            nc.vector.memset(ones64[:], 1.0)

            b_acc = cpool.tile([128, NI * L], F32, tag="b_acc")

            def w2_i(i):
                return w2[:, i * K * F:(i + 1) * K * F]

            for rep in range(repeat):
             for t in range(ITERS):
                # ---- coupling coefficients (unnormalized) ----
                if t == 0:
                    e9 = None         # e == 1 -> Wc == w2, invS == 1/1152
                    invS_b = None
                else:
                    # e replicated over O (contiguous innermost for Wc mult)
                    e9 = wpool.tile([128, NI * F], DT, tag="e9")
                    nc.scalar.activation(
                        e9[:].rearrange("p (i l o) -> p i l o", i=NI, l=L),
                        b_acc[:].rearrange("p (i l) -> p i l", i=NI)
                        .unsqueeze(3).to_broadcast((128, NI, L, O)),
                        AF.Exp, scale=1.0 / B)
                    # compact e for the column-sum S (N<=512 matmul limit)
                    e_nat = wpool.tile([128, NI * L], DT, tag="e_nat")
                    nc.scalar.activation(e_nat[:], b_acc[:], AF.Exp, scale=1.0 / B)
                    sm_ps = ps_sm.tile([BL, NI * L + L], F32, tag="sm")
                    ssum = sm_ps[0:1, 0:NI * L]
                    nc.tensor.matmul(ssum, ones[:], e_nat[:], start=True, stop=True)
                    S16 = spool.tile([1, L], F32, tag="S16")
                    nc.vector.tensor_reduce(
                        S16[:], ssum.rearrange("p (i l) -> p l i", i=NI),
                        axis=AX.X, op=ALU.add)
                    invS16 = spool.tile([1, L], F32, tag="invS16")
                    nc.vector.reciprocal(invS16[:], S16[:])
                    bc_ps = sm_ps[0:BL, NI * L:NI * L + L]
                    nc.tensor.matmul(bc_ps, ones64[:], invS16[:],
                                     start=True, stop=True)
                    invS_b = spool.tile([BL, L], F32, tag="invS_b")
                    nc.vector.tensor_copy(invS_b[:], bc_ps)

                # ---- s = x @ (e*w) ----
                s_ps = ps_s.tile([BL, F], F32, tag="s_ps")
                if "no_smm" in abl:
                    zz = spool.tile([BL, F], F32, tag="zz")
                    nc.vector.memset(zz[:], 0.0)
                    nc.scalar.activation(s_ps[:], zz[:], AF.Copy)
                wcs = []
                for i in range(NI):
                    if "no_smm" in abl:
                        break
                    if t == 0 or "no_wc" in abl:
                        wcs.append(w2_i(i))
                    else:
                        wc = wcfull.tile([128, K * F], DT, tag="wc")
                        e_b = (e9[:, i * F:(i + 1) * F]
                               .unsqueeze(1).to_broadcast((128, K, F)))
                        nc.vector.tensor_tensor(
                            wc[:].rearrange("p (k f) -> p k f", k=K),
                            w2_i(i).rearrange("p (k f) -> p k f", k=K),
                            e_b, op=ALU.mult)
                        wcs.append(wc[:])
                # k-outer matches the xT DMA arrival order (t = k*9+i)
                if "no_smm" not in abl:
                    for k in range(K):
                        for i in range(NI):
                            tslot = k * NI + i
                            nc.tensor.matmul(
                                s_ps[:],
                                xT[:, tslot * BL:(tslot + 1) * BL],
                                wcs[i][:, k * F:(k + 1) * F],
                                start=(i == 0 and k == 0),
                                stop=(i == NI - 1 and k == K - 1))

                # ---- squash (with 1/S[l] folded in) ----
                s_n = wpool.tile([BL, F], F32, tag="s_n")
                if t == 0:
                    nc.vector.tensor_scalar_mul(s_n[:], s_ps[:], 1.0 / I)
                else:
                    nc.vector.tensor_tensor(
                        s_n[:].rearrange("p (l o) -> p l o", o=O),
                        s_ps[:].rearrange("p (l o) -> p l o", o=O),
                        invS_b[:].unsqueeze(2).to_broadcast((BL, L, O)),
                        op=ALU.mult)
                # squash factor: sq/((1+sq)*sqrt(sq)) == sqrt(sq)/(1+sq)
                sq2 = wpool.tile([BL, F], F32, tag="sq2")
                nc.vector.tensor_tensor(sq2[:], s_n[:], s_n[:], op=ALU.mult)
                sq = spool.tile([BL, L], F32, tag="sq")
                nc.vector.tensor_reduce(
                    sq[:], sq2[:].rearrange("p (l o) -> p l o", o=O),
                    axis=AX.X, op=ALU.add)
                nrm = spool.tile([BL, L], F32, tag="nrm")
                nc.scalar.activation(nrm[:], sq[:], AF.Sqrt)
                d1 = spool.tile([BL, L], F32, tag="d1")
                nc.vector.tensor_scalar_add(d1[:], sq[:], 1.0)
                rin = spool.tile([BL, L], F32, tag="rin")
                nc.vector.reciprocal(rin[:], d1[:])
                fm = spool.tile([BL, L], F32, tag="fm")
                nc.vector.tensor_tensor(fm[:], nrm[:], rin[:], op=ALU.mult)
                # v in fp16 feeds the T2 matmuls directly; final iter in fp32
                vdt = F32 if t == ITERS - 1 else DT
                v_sb = wpool.tile([BL, F], vdt, tag="v_sb")
                nc.vector.tensor_tensor(
                    v_sb[:].rearrange("p (l o) -> p l o", o=O),
                    s_n[:].rearrange("p (l o) -> p l o", o=O),
                    fm[:].unsqueeze(2).to_broadcast((BL, L, O)),
                    op=ALU.mult)

                if t == ITERS - 1:
                    v_out = wpool.tile([BL, F], F32, tag="v_out")
                    nc.vector.tensor_copy(
                        v_out[:].rearrange("p (o l) -> p o l", l=L),
                        v_sb[:].rearrange("p (l o) -> p o l", o=O))
                    nc.sync.dma_start(y_d[:], v_out[:])
                    continue

                # ---- agreement: u = sum_{k,O} w * (x^T v) ----
                if "no_u" in abl:
                    if t == 0:
                        nc.vector.memset(b_acc[:], 0.0)
                    continue
                v16 = v_sb
                # agreement tensor in DT (fp16 halves AR payload + bounce DMAs)
                u_nat = wpool.tile([128, NI * L], DT, tag="u_nat")
                npool = 3
                for a in abl:
                    if a.startswith("pool"):
                        npool = int(a[4:])
                direct = "direct" in abl
                for i in range(NI):
                    prod = wcpool.tile([128, K * F], DT, tag="prod")
                    t2s = None if direct else wcpool.tile(
                        [128, K * F], DT, tag="t2s")
                    for h in range(2):
                        t2h = ps_t2.tile([128, 512], F32, tag="t2")
                        for kk in range(4):
                            k = h * 4 + kk
                            nc.tensor.matmul(
                                t2h[:, kk * 128:kk * 128 + F],
                                x_nat[:, k * I + i * 128:k * I + (i + 1) * 128],
                                v16[:], start=True, stop=True)
                        t2hv = (t2h[:].rearrange("p (k x) -> p k x", k=4)
                                [:, :, 0:F])
                        if direct:
                            nc.vector.tensor_tensor(
                                prod[:, h * 4 * F:(h + 1) * 4 * F]
                                .rearrange("p (k f) -> p k f", k=4),
                                t2hv,
                                w2_i(i)[:, h * 4 * F:(h + 1) * 4 * F]
                                .rearrange("p (k f) -> p k f", k=4),
                                op=ALU.mult)
                        else:
                            nc.scalar.activation(
                                t2s[:, h * 4 * F:(h + 1) * 4 * F]
                                .rearrange("p (k f) -> p k f", k=4),
                                t2hv, AF.Copy)
                    if not direct:
                        eng = nc.gpsimd if i < npool else nc.vector
                        eng.tensor_tensor(prod[:], t2s[:], w2_i(i), op=ALU.mult)
                    with nc.allow_low_precision("fp16 agreement; b re-acc fp32"):
                        nc.vector.tensor_reduce(
                            u_nat[:, i * L:(i + 1) * L],
                            prod[:].rearrange("p (k l o) -> p l k o", k=K, l=L),
                            axis=AX.XY, op=ALU.add)

                ar_in = dpool.tile([128, NI * L], DT, tag="ar_in")
                ar_out = dpool.tile([128, NI * L], DT, tag="ar_out")
                # bounce in thirds so the DMA overlaps the tail reduces
                for h in range(3):
                    lo, hi = h * 3 * L, (h + 1) * 3 * L
                    nc.sync.dma_start(ar_in[:, lo:hi], u_nat[:, lo:hi])
                if "no_ar" in abl:
                    nc.sync.dma_start(ar_out[:], ar_in[:])
                else:
                    nc.gpsimd.collective_compute(
                        "AllReduce", ALU.add,
                        replica_groups=[list(range(NC))],
                        ins=[ar_in.opt()], outs=[ar_out.opt()])
                u_allr = wpool.tile([128, NI * L], DT, tag="u_allr")
                nc.sync.dma_start(u_allr[:], ar_out[:])
                if t == 0:
                    nc.vector.tensor_copy(b_acc[:], u_allr[:])
                else:
                    nc.vector.tensor_add(b_acc[:], b_acc[:], u_allr[:])

    nc.compile()
    return nc


def _get_nc(dt_key, repeat=1, abl=()):
    key = (dt_key, repeat, tuple(sorted(abl)))
    if key not in _CACHE:
        _CACHE[key] = _build(dt_key, repeat, abl)
    return _CACHE[key]


def kernel(x, w, _dt="f16", _trace=False):
    x = np.asarray(x, dtype=np.float32)
    w = np.asarray(w, dtype=np.float32)
    np_dt = {"f32": np.float32, "f16": np.float16}[_dt]

    nc = _get_nc(_dt)

    w2 = np.ascontiguousarray(w.reshape(I, K * F).astype(np_dt))
    in_maps = []
    for c in range(NC):
        xv = x[c * BL:(c + 1) * BL]
        x_nat = np.ascontiguousarray(
            xv.transpose(0, 2, 1).astype(np_dt)).reshape(BL, R)
        x_T = np.ascontiguousarray(
            xv.transpose(2, 1, 0).astype(np_dt)).reshape(R, BL)
        in_maps.append({"x_nat": x_nat, "x_T": x_T, "w2": w2})

    from concourse.bass_utils import run_bass_kernel_spmd
    res = run_bass_kernel_spmd(
        nc, in_maps, core_ids=list(range(NC)), trace=_trace)
    kernel.last_result = res
    out = np.concatenate([res.results[c]["y"] for c in range(NC)], axis=0)
    return out.astype(np.float32)


kernel.last_result = None
